# revision 22
# baseline (speedup 1.0000x reference)
"""Grouped (MoE-style) linear on 8 trn2 NeuronCores.

out[t] = hidden_states[t] @ weight[g(t)], where token t belongs to group g iff
offsets[g-1] <= t < offsets[g] (searchsorted right semantics; tokens at or past
offsets[-1] get zero output).

Strategy: expert-parallel. Core g owns weight[g] and the contiguous token run
of group g. Routing is done host-side (offsets are host data); each core runs
an identical Bass program: [ntb*128, 1024] x [1024, 1024] matmul in bf16
(inputs cast host-side; PSUM accumulation stays fp32, measured 2.3e-3 relmax
for this distribution -- well inside the 2e-2 gate).

Performance notes (all measured on hw via NTFF traces):
  - The kernel is tensor-bound: 256 N=512 bf16 matmuls/core = 55.3us at the
    216ns warm back-to-back rate (fp32r was 244ns -- no fast weight load).
  - The PE clock-gate (HAM) starts at 1.2 GHz and releases to 2.4 GHz only
    after ~5-6us of GAP-FREE matmul activity; any ramp stall resets the
    window (one 2.5us stall measured as ~4.5us of extra cold time).  So the
    loop is structured so the ramp never stalls: group 0 covers THREE token
    blocks (cold k-steps = 6 MMs = 2.6us, slower than the DMA stream), and
    every DMA is issued on ONE ring in exact consumption order -- the two
    HWDGE rings share the 16 SDMA engines ~50/50, so a second-ring prefetch
    stream starves the critical one (measured twice).
  - Every DMA semaphore fires ~2.3us after its last byte (HBM write-receipt
    round trip); the ordering below keeps each k-step's data a step ahead
    of the PE despite that.
  - Outputs drain progressively per block (ACT copies PSUM[0:512] -> sync
    ring; DVE copies PSUM[512:] -> scalar ring), so the post-loop tail is
    one copy + 2x256KB DMA + receipt (~4.5us), and a fixed ~8.7us framework
    epilogue (254 individual semaphore clears + barrier) follows.
"""
import numpy as np
import ml_dtypes

import concourse.bass as bass
import concourse.tile as tile
from concourse import bacc, mybir
from concourse.bass_utils import run_bass_kernel_spmd

GROUPS = 8
TOKENS = 16384
IN_F = 1024
OUT_F = 1024
KCH = IN_F // 128  # contraction chunks
BF16 = ml_dtypes.bfloat16


def _group_tbs(ntb: int) -> list[list[int]]:
    """Token-block groups: [3, 2, 2, ..., 1] when possible.

    A wide first group keeps the cold-clock ramp gap-free; a single-block
    last group minimizes the post-loop drain. Each group's PSUM tiles
    (2 banks per block) cycle through the 4-buffer pool.
    """
    if ntb <= 3:
        return [list(range(ntb))]
    tbs = list(range(ntb))
    groups = [tbs[0:3]]
    rest = tbs[3:]
    while len(rest) > 2:
        groups.append(rest[0:2])
        rest = rest[2:]
    while rest:
        groups.append([rest.pop(0)])
    return groups


def build(ntb: int) -> bass.Bass:
    """One core's program: ntb 128-token blocks through a 1024x1024 expert."""
    f32 = mybir.dt.float32
    bf16 = mybir.dt.bfloat16
    n0 = min(ntb, 3)  # blocks in the first group
    nc = bacc.Bacc()
    # xt[tb, p, k, tok] = X[tb*128 + tok, k*128 + p]
    xt_d = nc.dram_tensor("xt", [ntb, 128, KCH, 128], bf16,
                          kind="ExternalInput")
    # boot fuses group 0's k=0 X chunk and w[k0]'s first half into ONE
    # contiguous tensor: the first matmul then gates on a single DMA
    # completion (each completion costs a ~2us HBM-receipt round trip, so
    # two serialized bootstrap DMAs would start the PE ~1.3us later)
    boot_d = nc.dram_tensor("boot", [128, n0 * 128 + 512], bf16,
                            kind="ExternalInput")
    # w[k, p, n] = W[k*128 + p, n]
    w_d = nc.dram_tensor("w", [KCH, 128, OUT_F], bf16, kind="ExternalInput")
    out_d = nc.dram_tensor("out", [ntb * 128, OUT_F], f32,
                           kind="ExternalOutput")
    groups = _group_tbs(ntb)

    with tile.TileContext(nc) as tc:
        with (
            tc.tile_pool(name="wp", bufs=1) as wp,
            tc.tile_pool(name="xp", bufs=min(ntb, 40)) as xp,
            tc.tile_pool(name="op", bufs=4) as op,
            tc.tile_pool(name="ps", bufs=4, space="PSUM") as psp,
        ):
            wt = wp.tile([128, KCH, OUT_F], bf16)
            boot = wp.tile([128, n0 * 128 + 512], bf16)
            # single input ring, exact consumption order
            nc.sync.dma_start(out=boot[:], in_=boot_d[:])
            nc.sync.dma_start(out=wt[:, 0, 512:], in_=w_d[0, :, 512:])
            nc.sync.dma_start(out=wt[:, 1, :], in_=w_d[1])
            xts = []
            for tb in range(n0):
                xtn = xp.tile([128, KCH, 128], bf16, tag="xt")
                nc.sync.dma_start(out=xtn[:], in_=xt_d[tb])
                xts.append(xtn)
            for k in range(2, KCH):
                nc.sync.dma_start(out=wt[:, k, :], in_=w_d[k])
            for tb in range(n0, ntb):
                xtn = xp.tile([128, KCH, 128], bf16, tag="xt")
                nc.sync.dma_start(out=xtn[:], in_=xt_d[tb])
                xts.append(xtn)

            for gi, tbs in enumerate(groups):
                pss = [psp.tile([128, OUT_F], f32, name="ps", tag="ps")
                       for _ in tbs]
                for k in range(KCH):
                    for j, ps in enumerate(pss):
                        first = gi == 0 and k == 0
                        stat = boot[:, j * 128:(j + 1) * 128] if first \
                            else xts[tbs[j]][:, k, :]
                        for nh in range(2):
                            # w[k0] nh0 lives only in boot (never in wt)
                            rhs = boot[:, n0 * 128:] if (k == 0 and nh == 0) \
                                else wt[:, k, nh * 512:(nh + 1) * 512]
                            nc.tensor.matmul(
                                ps[:, nh * 512:(nh + 1) * 512],
                                stat,
                                rhs,
                                start=(k == 0),
                                stop=(k == KCH - 1),
                            )
                for j, ps in enumerate(pss):
                    tb = tbs[j]
                    ot = op.tile([128, OUT_F], f32)
                    rows = slice(tb * 128, (tb + 1) * 128)
                    nc.scalar.copy(ot[:, 0:512], ps[:, 0:512])
                    nc.sync.dma_start(out=out_d[rows, 0:512],
                                      in_=ot[:, 0:512])
                    nc.vector.tensor_copy(ot[:, 512:], ps[:, 512:])
                    nc.scalar.dma_start(out=out_d[rows, 512:],
                                        in_=ot[:, 512:])
    nc.compile()
    return nc


def _pack_core(x_slice: np.ndarray, w_g: np.ndarray, ntb: int):
    n = x_slice.shape[0]
    n0 = min(ntb, 3)
    xp = np.zeros((ntb * 128, IN_F), dtype=np.float32)
    xp[:n] = x_slice
    # [tb, tok, k, p] -> [tb, p, k, tok]
    xt = np.ascontiguousarray(
        xp.reshape(ntb, 128, KCH, 128).transpose(0, 3, 2, 1).astype(BF16)
    )
    wt = np.ascontiguousarray(w_g.reshape(KCH, 128, OUT_F).astype(BF16))
    x0 = xt[0:n0, :, 0, :].transpose(1, 0, 2).reshape(128, n0 * 128)
    boot = np.ascontiguousarray(
        np.concatenate([x0, wt[0, :, 0:512]], axis=1)
    )
    return xt, boot, wt


def kernel(hidden_states: np.ndarray, weight: np.ndarray, offsets: np.ndarray,
           _trace: bool = False):
    hs = np.ascontiguousarray(hidden_states, dtype=np.float32)
    w = np.ascontiguousarray(weight, dtype=np.float32)
    off = np.asarray(offsets).astype(np.int64)

    ends = np.clip(off, 0, TOKENS)
    starts = np.concatenate(([0], ends[:-1]))
    starts = np.minimum(starts, ends)
    ns = ends - starts

    ntb = max(1, int(-(-ns.max() // 128)))
    nc = build(ntb)

    in_maps = []
    for g in range(GROUPS):
        xt, boot, wt = _pack_core(hs[starts[g]:ends[g]], w[g], ntb)
        in_maps.append({"xt": xt, "boot": boot, "w": wt})

    res = run_bass_kernel_spmd(nc, in_maps, list(range(GROUPS)), trace=_trace)

    out = np.zeros((TOKENS, OUT_F), dtype=np.float32)
    for g in range(GROUPS):
        if ns[g] > 0:
            out[starts[g]:ends[g]] = res.results[g]["out"][:ns[g]]
    if _trace:
        return out, res
    return out
